# revision 6
# baseline (speedup 1.0000x reference)
"""Memristive fully-connected layer on 8 Trainium2 NeuronCores (bf16).

The reference's differential conductance pair collapses algebraically:
g_pos - g_neg = g_eff = k_cond * weights, and the final rescale divides
K_V * k_cond back out, so the module computes exactly y = x @ w + b.

Strategy: data-parallel over the batch, inputs cast to bf16 on host.
bf16 matmuls run at the same 1 cycle/moving-row PE rate as float32r
(steady-state cadence measured 216 ns per 512-wide matmul = the PE
floor), while halving all DMA traffic and letting FWL hide LDWEIGHTS.

Per core: 8 n-blocks of 512 columns (one PSUM bank each); the
contraction sweeps 32 k-tiles with all 8 output-row tiles per k-tile,
so each PSUM bank's final matmul sits well ahead of the next block's
first use. Bias is broadcast across partitions once and added on PSUM
eviction by the vector engine; y is written as bf16 (host upcasts).

Startup is the critical region (the steady state runs at the PE floor):
- warm-tile memset on the vector engine (gpsimd pays ~7 us dispatch),
  then 26 throwaway matmuls keep the PE busy without gaps until real
  data lands, so the HAM clock-gate opens to 2.4 GHz right as the real
  stream begins;
- the first two k-blocks of w and xT transfer per-k-tile (128/256 KB),
  interleaved in exact consumption order on the two HWDGE rings, and
  those k-blocks run k-outer/m-inner so one k-tile is consumed per
  ~1.7 us - matching the 8-core-contended HBM arrival cadence instead
  of demanding 4 k-tiles in the first 4 matmuls;
- the rest stream as 512 KB / 1 MB batches (many small transfers lose
  to the ~2 us per-transfer completion latency on the 8 DMA lanes).
The tail's last output tile is evicted and stored in halves across
both DMA rings so the two store receipts overlap.
"""

import numpy as np
import ml_dtypes

import concourse.bass as bass  # noqa: F401  (registers engine classes)
import concourse.mybir as mybir
from concourse import bacc, tile
from concourse.bass_utils import run_bass_kernel_spmd

dt = mybir.dt

BATCH, N_IN, N_OUT = 8192, 4096, 4096
NCORES = 8
MB = BATCH // NCORES          # 1024 batch rows per core
P = 128
KT = N_IN // P                # 32 contraction tiles
MT = MB // P                  # 8 output-row tiles per core
NBLK = 512                    # matmul free dim (one PSUM bank)
NB = N_OUT // NBLK            # 8 output-column blocks
KB = 4                        # k-tiles per w-block DMA (nb >= 1)
NKB = KT // KB                # 8 k-blocks
WARMUP_MM = 26
FINE_KB = 2                   # leading k-blocks with per-k-tile DMAs

_cache = {}


def _build():
    nc = bacc.Bacc("TRN2", target_bir_lowering=False, debug=False)
    xT = nc.dram_tensor("xT", [N_IN, MB], dt.bfloat16, kind="ExternalInput")
    w = nc.dram_tensor("w", [N_IN, N_OUT], dt.bfloat16, kind="ExternalInput")
    b = nc.dram_tensor("b", [1, N_OUT], dt.float32, kind="ExternalInput")
    y = nc.dram_tensor("y", [MB, N_OUT], dt.bfloat16, kind="ExternalOutput")

    xT_r = xT.rearrange("(kt p) m -> p kt m", p=P)    # [128, 32, 1024]
    w_r = w.rearrange("(kt p) n -> p kt n", p=P)      # [128, 32, 4096]
    y_r = y.rearrange("(mt p) n -> p mt n", p=P)      # [128, 8, 4096]

    with tile.TileContext(nc) as tc:
        with (
            tc.tile_pool(name="xtp", bufs=1) as xtp,
            tc.tile_pool(name="wsp", bufs=8) as wsp,
            tc.tile_pool(name="wp", bufs=8) as wp,
            tc.tile_pool(name="bp", bufs=1) as bp,
            tc.tile_pool(name="op", bufs=6) as op,
            tc.tile_pool(name="ps", bufs=1, space="PSUM") as ps,
        ):
            # HAM warmup: throwaway matmuls on a vector-memset tile while
            # the first DMAs are in flight. Vector dispatch is fast, so
            # the PE is busy within ~0.5 us of kernel start.
            warm = bp.tile([P, 256], dt.bfloat16, name="warm")
            nc.vector.memset(warm[:], 0.0)
            wpsums = [
                ps.tile([P, NBLK], dt.float32, name=f"ps{i}") for i in range(MT)
            ]
            for i in range(WARMUP_MM):
                nc.tensor.matmul(
                    wpsums[i % MT][:, :256], warm[:, :P], warm[:],
                    start=True, stop=True,
                )

            # w k-block DMA: 4 k-tiles (512 KB) per transfer on the SP ring.
            def w_dma(nb, kb):
                wt = wp.tile([P, KB, NBLK], dt.bfloat16, name="wt")
                nc.sync.dma_start(
                    wt[:],
                    w_r[:, kb * KB:(kb + 1) * KB, nb * NBLK:(nb + 1) * NBLK],
                )
                return [wt[:, kk, :] for kk in range(KB)]

            # Startup transfers in exact consumption order: w(nb=0) rides
            # the SP ring, xT the Activation ring. Only the first k-block
            # is per-k-tile (128/256 KB) so the first matmul's data lands
            # fast; the rest are k-block batches (512 KB / 1 MB) — the
            # ~2 us per-transfer completion latency on the 8 DMA lanes
            # makes many small transfers drain slower than few big ones.
            xts = xtp.tile([P, KT, MB], dt.bfloat16, name="xts")
            w0 = [None] * KT
            for k in range(FINE_KB * KB):
                wt = wsp.tile([P, NBLK], dt.bfloat16, name="wst")
                nc.sync.dma_start(wt[:], w_r[:, k, 0:NBLK])
                w0[k] = wt
                nc.scalar.dma_start(xts[:, k, :], xT_r[:, k, :])
            wblk0 = [None] * NKB
            for kb in range(FINE_KB, NKB):
                wblk0[kb] = w_dma(0, kb)
                nc.scalar.dma_start(
                    xts[:, kb * KB:(kb + 1) * KB, :],
                    xT_r[:, kb * KB:(kb + 1) * KB, :],
                )
            for kb in range(FINE_KB, NKB):
                for kk in range(KB):
                    w0[kb * KB + kk] = wblk0[kb][kk]

            # Bias: DMA the row into partition 0, broadcast in place on
            # gpsimd (idle otherwise); first needed at ~55 us.
            bias_sb = bp.tile([P, N_OUT], dt.float32, name="bias_sb")
            nc.scalar.dma_start(bias_sb[0:1, :], b[:, :])
            nc.gpsimd.partition_broadcast(bias_sb[:], bias_sb[0:1, :])

            for nb in range(NB):
                psums = [
                    ps.tile([P, NBLK], dt.float32, name=f"ps{m}")
                    for m in range(MT)
                ]
                ot = None
                for kb in range(NKB):
                    if nb == 0:
                        wts = [w0[kb * KB + kk] for kk in range(KB)]
                    else:
                        wts = w_dma(nb, kb)
                    if nb == 0 and kb < FINE_KB:
                        # The leading k-blocks' tiles arrive one per ~1.7 us.
                        # k-outer/m-inner consumes one k-tile per 8 matmuls,
                        # matching the arrival cadence; the default m-outer
                        # order would need all 4 k-tiles within 4 matmuls
                        # and stall the PE into a HAM re-throttle.
                        for kk in range(KB):
                            k = kb * KB + kk
                            for m in range(MT):
                                nc.tensor.matmul(
                                    psums[m][:],
                                    xts[:, k, m * P:(m + 1) * P],
                                    wts[kk],
                                    start=(k == 0),
                                    stop=False,
                                )
                        continue
                    for m in range(MT):
                        for kk in range(KB):
                            k = kb * KB + kk
                            nc.tensor.matmul(
                                psums[m][:],
                                xts[:, k, m * P:(m + 1) * P],
                                wts[kk],
                                start=(k == 0),
                                stop=(k == KT - 1),
                            )
                        if kb == NKB - 1:
                            if nb == NB - 1:
                                # final block: single-tile stores alternated
                                # across both DMA rings shorten the tail
                                ot = op.tile([P, 2, NBLK], dt.bfloat16, name="ot")
                                if m == MT - 1:
                                    # last tile is the critical path: evict
                                    # and store in halves so the first store
                                    # issues ~0.35 us earlier and the two
                                    # receipts overlap on both rings
                                    H = NBLK // 2
                                    nc.vector.tensor_add(
                                        ot[:, 0, 0:H],
                                        psums[m][:, 0:H],
                                        bias_sb[:, nb * NBLK:nb * NBLK + H],
                                    )
                                    nc.sync.dma_start(
                                        y_r[:, m:m + 1, nb * NBLK:nb * NBLK + H],
                                        ot[:, 0:1, 0:H],
                                    )
                                    nc.vector.tensor_add(
                                        ot[:, 0, H:NBLK],
                                        psums[m][:, H:NBLK],
                                        bias_sb[:, nb * NBLK + H:(nb + 1) * NBLK],
                                    )
                                    nc.scalar.dma_start(
                                        y_r[:, m:m + 1, nb * NBLK + H:(nb + 1) * NBLK],
                                        ot[:, 0:1, H:NBLK],
                                    )
                                else:
                                    nc.vector.tensor_add(
                                        ot[:, 0, :],
                                        psums[m][:],
                                        bias_sb[:, nb * NBLK:(nb + 1) * NBLK],
                                    )
                                    eng = nc.scalar if m % 2 else nc.sync
                                    eng.dma_start(
                                        y_r[:, m:m + 1, nb * NBLK:(nb + 1) * NBLK],
                                        ot[:, 0:1, :],
                                    )
                            else:
                                if m % 2 == 0:
                                    ot = op.tile([P, 2, NBLK], dt.bfloat16, name="ot")
                                nc.vector.tensor_add(
                                    ot[:, m % 2, :],
                                    psums[m][:],
                                    bias_sb[:, nb * NBLK:(nb + 1) * NBLK],
                                )
                                if m % 2 == 1:
                                    nc.scalar.dma_start(
                                        y_r[:, m - 1:m + 1, nb * NBLK:(nb + 1) * NBLK],
                                        ot[:],
                                    )
    nc.compile()
    return nc


def kernel(x, w, b, _trace=False, _trace_kwargs=None):
    if "nc" not in _cache:
        _cache["nc"] = _build()
    nc = _cache["nc"]

    b2 = np.ascontiguousarray(np.asarray(b, dtype=np.float32).reshape(1, N_OUT))
    w2 = np.ascontiguousarray(
        np.asarray(w, dtype=np.float32).astype(ml_dtypes.bfloat16)
    )
    in_maps = []
    for c in range(NCORES):
        xs = np.ascontiguousarray(
            x[c * MB:(c + 1) * MB].T.astype(ml_dtypes.bfloat16)
        )
        in_maps.append({"xT": xs, "w": w2, "b": b2})

    res = run_bass_kernel_spmd(
        nc,
        in_maps,
        core_ids=list(range(NCORES)),
        trace=_trace,
        **(_trace_kwargs or {}),
    )
    out = np.concatenate(
        [np.asarray(res.results[c]["y"]).astype(np.float32) for c in range(NCORES)],
        axis=0,
    )
    if _trace:
        return out, res
    return out
